# revision 9
# baseline (speedup 1.0000x reference)
"""Trainium2 Bass kernel for the merged multi-adapter LoRA layer.

Math (all fp32 reference):
    t[n,b,j,d]  = sum_m x[b,j,m] * lora_A[n,d,m]
    out[n,b,j,k] = sum_d t[n,b,j,d] * lora_B[n,k,d]

Shapes: x (4,2048,4096), lora_A (4,16,4096), lora_B (4,4096,16)
        out (4,4,2048,4096)

Sharding: data-parallel over flattened tokens (b*j = 8192 -> 1024/core on
8 cores); the tiny LoRA params are replicated.

Per-core HBM traffic: 8 MiB x (f16 in) + 32 MiB out (f16, widened on host)
+ ~2 MiB params  ->  ~117 us at 358 GB/s.  The schedule is built around the
PE HAM clock-gate: the PE runs at 2.4 GHz only while it is continuously
busy, and at 1.2 GHz otherwise, so the kernel sprints (PE gap-free, deep
output runway) rather than pacing itself to the store rate.

Layout / schedule notes:
  - x arrives pre-transposed/packed as [chunk, half, 128, 8, 512] f16; one
    1 MiB DMA per half-chunk, split across the Scalar and Sync trigger
    queues so the first chunk lands ~3 us after the body starts.
  - warm-up matmuls on a zeroed scratch tile (no DMA dependency) raise the
    HAM to 2.4 GHz while the first x chunk streams in.
  - mm1 (chunk c) is a back-to-back 32-matmul accumulation chain; between
    chunks it doubles as a HAM re-warmer.  mm2 produces [128,4096] f16
    output strips via 8 x 512-wide matmuls per strip, adapters on their
    own 32-row PE tile_position.
  - PSUM->SBUF f32->f16 evacuation: 1024-wide copies alternating
    Vector/Scalar (1024-wide amortizes the ~160 ns per-op ramp); a
    3-deep [128,1024] PSUM ring keeps the PE from blocking on copies.
  - 13 staged output strips in SBUF let the warm PE run far ahead of the
    store wire, so stores stay saturated after the HAM eventually
    re-throttles.
"""

import numpy as np

import concourse.bacc as bacc
import concourse.bass as bass
import concourse.mybir as mybir
import concourse.tile as tile
from concourse import bass_utils
from concourse.bass import ds, ts

F32 = mybir.dt.float32
F16 = mybir.dt.float16

N_CORES = 8
B, J, M = 4, 2048, 4096
N, D, K = 4, 16, 4096
TOK = B * J                  # 8192 flattened tokens
TPC = TOK // N_CORES         # 1024 tokens per core
CH = 256                     # token chunk (mm1 granularity)
NCH = TPC // CH              # 4
N_MT = M // 128              # 32 m-tiles
NPAIR = N_MT // 2            # 16 packed m-tile pairs
NPH = NPAIR // 2             # pairs per half-chunk DMA (8)
KT = 512                     # mm2 matmul free width (one PSUM bank)
OPW = 1024                   # PSUM evacuation width (two banks)
ADP = 32                     # partition stride per adapter in the packed dim
NSTRIP = CH // 128           # 128-token strips per chunk (2)
WARMUP = 10                  # scratch matmuls to un-throttle the PE HAM


def build_program():
    nc = bacc.Bacc("TRN2")

    xs = nc.dram_tensor(
        "xs", [NCH, 2, 128, NPH, 2 * CH], F16, kind="ExternalInput"
    ).ap()
    a_p = nc.dram_tensor("a_p", [128, N_MT, 128], F16, kind="ExternalInput").ap()
    b_p = nc.dram_tensor("b_p", [128, K], F16, kind="ExternalInput").ap()
    o = nc.dram_tensor("o", [N, TPC, K], F16, kind="ExternalOutput").ap()

    with tile.TileContext(nc) as tc:
        with (
            tc.tile_pool(name="apool", bufs=1) as apool,
            tc.tile_pool(name="bpool", bufs=1) as bpool,
            tc.tile_pool(name="spool", bufs=1) as spool,
            tc.tile_pool(name="xpool", bufs=2 * NCH) as xpool,
            tc.tile_pool(name="tpool", bufs=2) as tpool,
            tc.tile_pool(name="opool", bufs=13) as opool,
            tc.tile_pool(name="tps", bufs=1, space="PSUM") as tps_pool,
            tc.tile_pool(name="ops", bufs=3, space="PSUM") as ops_pool,
            tc.tile_pool(name="fps", bufs=1, space="PSUM") as fps_pool,
        ):
            # x half-chunks: first chunk on Scalar's queue, a/b and the
            # back chunks on Sync's (it has no stores to issue yet)
            xsb = {}
            for c in range(NCH):
                for h in range(2):
                    xsb[(c, h)] = xpool.tile([128, NPH, 2 * CH], F16, tag="x", name="x")
            a_sb = apool.tile([128, N_MT, 128], F16, tag="a")
            b_sb = bpool.tile([128, K], F16, tag="b")

            nc.scalar.dma_start(xsb[(0, 0)][:], xs[0, 0])
            nc.sync.dma_start(a_sb[:], a_p[:])
            nc.scalar.dma_start(xsb[(0, 1)][:], xs[0, 1])
            nc.sync.dma_start(b_sb[:], b_p[:])
            nc.scalar.dma_start(xsb[(1, 0)][:], xs[1, 0])
            nc.sync.dma_start(xsb[(2, 0)][:], xs[2, 0])
            nc.scalar.dma_start(xsb[(1, 1)][:], xs[1, 1])
            nc.sync.dma_start(xsb[(2, 1)][:], xs[2, 1])
            nc.scalar.dma_start(xsb[(3, 0)][:], xs[3, 0])
            nc.scalar.dma_start(xsb[(3, 1)][:], xs[3, 1])

            scr = spool.tile([128, KT], F16, tag="s", name="scr")
            nc.vector.memset(scr[:], 0.0)
            f_sc = fps_pool.tile([128, KT], F32, tag="f", name="f")

            def filler():
                nc.tensor.matmul(f_sc[:], lhsT=scr[:, ds(0, 128)], rhs=scr[:],
                                 start=True, stop=True, skip_group_check=True)

            # HAM warm-up while the first x chunk streams in
            for _ in range(WARMUP):
                filler()

            def mm1(c, mt, t_ps):
                nc.tensor.matmul(
                    t_ps[:],
                    lhsT=a_sb[:, mt, :],
                    rhs=xsb[(c, mt // 16)][:, (mt // 2) % NPH, ds((mt % 2) * CH, CH)],
                    start=(mt == 0),
                    stop=(mt == N_MT - 1),
                    skip_group_check=True,
                )

            def mm1_chain(c):
                t_ps = tps_pool.tile([128, CH], F32, tag="tps", name="tps")
                for mt in range(N_MT):
                    mm1(c, mt, t_ps)
                t_sb = tpool.tile([128, CH], F16, tag="t", name="t")
                nc.vector.tensor_copy(t_sb[:], t_ps[:])
                return t_sb

            evac = 0
            t_sb_next = mm1_chain(0)

            for c in range(NCH):
                t_sb = t_sb_next

                for slot in range(NSTRIP * N):
                    s, n = divmod(slot, N)
                    if c == 0 and slot == 0:
                        filler()
                        filler()
                    osb = opool.tile([128, K], F16, tag="o", name="osb")
                    for kg in range(K // OPW):
                        o_ps = ops_pool.tile([128, OPW], F32, tag="ops", name="ops")
                        for kk in range(OPW // KT):
                            nc.tensor.matmul(
                                o_ps[:, ts(kk, KT)],
                                lhsT=t_sb[ds(ADP * n, D), ts(s, 128)],
                                rhs=b_sb[ds(ADP * n, D), ds(kg * OPW + kk * KT, KT)],
                                start=True,
                                stop=True,
                                tile_position=(ADP * n, 0),
                                skip_group_check=True,
                            )
                        if evac % 2 == 0:
                            nc.vector.tensor_copy(osb[:, ts(kg, OPW)], o_ps[:])
                        else:
                            nc.scalar.copy(osb[:, ts(kg, OPW)], o_ps[:])
                        evac += 1

                    nc.sync.dma_start(
                        o[n, ds(c * CH + s * 128, 128), :], osb[:]
                    )
                    # the next chunk's mm1 chain right after the first slot:
                    # late enough that its x has landed, early enough that
                    # the back-to-back chain re-warms the HAM mid-chunk
                    if slot == 0 and c + 1 < NCH:
                        t_sb_next = mm1_chain(c + 1)

    nc.compile()
    return nc


_NC_CACHE = []


def _get_nc():
    if not _NC_CACHE:
        _NC_CACHE.append(build_program())
    return _NC_CACHE[0]


def prepare_inputs(x, lora_A, lora_B):
    x = np.ascontiguousarray(np.asarray(x, dtype=np.float32)).astype(np.float16)
    lora_A = np.asarray(lora_A, dtype=np.float32)
    lora_B = np.asarray(lora_B, dtype=np.float32)

    xf = x.reshape(TOK, M)

    # a_t[m, 32n+d] = lora_A[n, d, m]; packed to [p, mt, c] so each SBUF
    # partition reads one contiguous row.
    a_t = np.zeros((M, 128), dtype=np.float32)
    for n in range(N):
        a_t[:, ADP * n : ADP * n + D] = lora_A[n].T
    a_pack = np.ascontiguousarray(
        a_t.reshape(N_MT, 128, 128).transpose(1, 0, 2)
    ).astype(np.float16)

    # b_pad[32n+d, k] = lora_B[n, k, d]
    b_pad = np.zeros((128, K), dtype=np.float16)
    for n in range(N):
        b_pad[ADP * n : ADP * n + D, :] = lora_B[n].T

    in_maps = []
    for c in range(N_CORES):
        # xp[chunk, half, p, pq, sub*CH + t] = x^T[(2*(8h+pq)+sub)*128 + p,
        #                                          chunk*CH + t]
        xT = xf[c * TPC : (c + 1) * TPC].T                  # [M, TPC]
        xr = xT.reshape(2, NPH, 2, 128, NCH, CH)            # [h, pq, sub, p, ch, t]
        xp = np.ascontiguousarray(xr.transpose(4, 0, 3, 1, 2, 5)).reshape(
            NCH, 2, 128, NPH, 2 * CH
        )
        in_maps.append({"xs": xp, "a_p": a_pack, "b_p": b_pad})
    return in_maps


def run(x, lora_A, lora_B, trace=False, **spmd_kwargs):
    nc = _get_nc()
    in_maps = prepare_inputs(x, lora_A, lora_B)
    res = bass_utils.run_bass_kernel_spmd(
        nc, in_maps, list(range(N_CORES)), trace=trace, **spmd_kwargs
    )
    o_full = np.concatenate(
        [res.results[c]["o"].astype(np.float32) for c in range(N_CORES)], axis=1
    )
    return o_full.reshape(N, B, J, K), res


def kernel(x, lora_A, lora_B):
    out, _ = run(x, lora_A, lora_B)
    return out


# revision 11
# speedup vs baseline: 1.0264x; 1.0264x over previous
"""Trainium2 Bass kernel for the merged multi-adapter LoRA layer.

Math (all fp32 reference):
    t[n,b,j,d]  = sum_m x[b,j,m] * lora_A[n,d,m]
    out[n,b,j,k] = sum_d t[n,b,j,d] * lora_B[n,k,d]

Shapes: x (4,2048,4096), lora_A (4,16,4096), lora_B (4,4096,16)
        out (4,4,2048,4096)

Sharding: data-parallel over flattened tokens (b*j = 8192 -> 1024/core on
8 cores); the tiny LoRA params are replicated.

Per-core HBM traffic: 8 MiB x (f16 in) + 32 MiB out (f16, widened on host)
+ ~2 MiB params  ->  ~117 us at 358 GB/s.  The schedule is built around the
PE HAM clock-gate: the PE runs at 2.4 GHz only while it is continuously
busy, and at 1.2 GHz otherwise, so the kernel sprints (PE gap-free, deep
output runway) rather than pacing itself to the store rate.

Layout / schedule notes:
  - x arrives pre-transposed/packed as [chunk, half, 128, 8, 512] f16; one
    1 MiB DMA per half-chunk, split across the Scalar and Sync trigger
    queues so the first chunk lands ~3 us after the body starts.
  - warm-up matmuls on a zeroed scratch tile (no DMA dependency) raise the
    HAM to 2.4 GHz while the first x chunk streams in.
  - mm1 (chunk c) is a back-to-back 32-matmul accumulation chain; between
    chunks it doubles as a HAM re-warmer.  mm2 produces [128,4096] f16
    output strips via 8 x 512-wide matmuls per strip, adapters on their
    own 32-row PE tile_position.
  - PSUM->SBUF f32->f16 evacuation: 1024-wide copies alternating
    Vector/Scalar (1024-wide amortizes the ~160 ns per-op ramp); a
    3-deep [128,1024] PSUM ring keeps the PE from blocking on copies.
  - 13 staged output strips in SBUF let the warm PE run far ahead of the
    store wire, so stores stay saturated after the HAM eventually
    re-throttles.
"""

import numpy as np

import concourse.bacc as bacc
import concourse.bass as bass
import concourse.mybir as mybir
import concourse.tile as tile
from concourse import bass_utils
from concourse.bass import ds, ts

F32 = mybir.dt.float32
F16 = mybir.dt.float16

N_CORES = 8
B, J, M = 4, 2048, 4096
N, D, K = 4, 16, 4096
TOK = B * J                  # 8192 flattened tokens
TPC = TOK // N_CORES         # 1024 tokens per core
CH = 256                     # token chunk (mm1 granularity)
NCH = TPC // CH              # 4
N_MT = M // 128              # 32 m-tiles
NPAIR = N_MT // 2            # 16 packed m-tile pairs
NPH = NPAIR // 2             # pairs per half-chunk DMA (8)
KT = 512                     # mm2 matmul free width (one PSUM bank)
OPW = 1024                   # PSUM evacuation width (two banks)
ADP = 32                     # partition stride per adapter in the packed dim
NSTRIP = CH // 128           # 128-token strips per chunk (2)
WARMUP = 16                  # scratch matmuls to un-throttle the PE HAM


def build_program():
    nc = bacc.Bacc("TRN2")

    xs = nc.dram_tensor(
        "xs", [NCH, 2, 128, NPH, 2 * CH], F16, kind="ExternalInput"
    ).ap()
    a_p = nc.dram_tensor("a_p", [128, N_MT, 128], F16, kind="ExternalInput").ap()
    b_p = nc.dram_tensor("b_p", [128, K], F16, kind="ExternalInput").ap()
    o = nc.dram_tensor("o", [N, TPC, K], F16, kind="ExternalOutput").ap()

    with tile.TileContext(nc) as tc:
        with (
            tc.tile_pool(name="apool", bufs=1) as apool,
            tc.tile_pool(name="bpool", bufs=1) as bpool,
            tc.tile_pool(name="spool", bufs=1) as spool,
            tc.tile_pool(name="xpool", bufs=2 * NCH) as xpool,
            tc.tile_pool(name="tpool", bufs=2) as tpool,
            tc.tile_pool(name="opool", bufs=13) as opool,
            tc.tile_pool(name="tps", bufs=1, space="PSUM") as tps_pool,
            tc.tile_pool(name="ops", bufs=3, space="PSUM") as ops_pool,
            tc.tile_pool(name="fps", bufs=1, space="PSUM") as fps_pool,
        ):
            # x half-chunks: first chunk on Scalar's queue, a/b and the
            # back chunks on Sync's (it has no stores to issue yet)
            xsb = {}
            for c in range(NCH):
                for h in range(2):
                    xsb[(c, h)] = xpool.tile([128, NPH, 2 * CH], F16, tag="x", name="x")
            a_sb = apool.tile([128, N_MT, 128], F16, tag="a")
            b_sb = bpool.tile([128, K], F16, tag="b")

            # the two halves of each chunk load in parallel on the two
            # hardware-DGE trigger queues (Scalar and Sync)
            nc.scalar.dma_start(xsb[(0, 0)][:], xs[0, 0])
            nc.sync.dma_start(a_sb[:], a_p[:])
            nc.sync.dma_start(xsb[(0, 1)][:], xs[0, 1])
            nc.scalar.dma_start(xsb[(1, 0)][:], xs[1, 0])
            nc.sync.dma_start(b_sb[:], b_p[:])
            nc.sync.dma_start(xsb[(1, 1)][:], xs[1, 1])
            nc.scalar.dma_start(xsb[(2, 0)][:], xs[2, 0])
            nc.sync.dma_start(xsb[(2, 1)][:], xs[2, 1])
            nc.scalar.dma_start(xsb[(3, 0)][:], xs[3, 0])
            nc.sync.dma_start(xsb[(3, 1)][:], xs[3, 1])

            scr = spool.tile([128, KT], F16, tag="s", name="scr")
            nc.vector.memset(scr[:], 0.0)
            f_sc = fps_pool.tile([128, KT], F32, tag="f", name="f")

            def filler():
                nc.tensor.matmul(f_sc[:], lhsT=scr[:, ds(0, 128)], rhs=scr[:],
                                 start=True, stop=True, skip_group_check=True)

            # HAM warm-up while the first x chunk streams in
            for _ in range(WARMUP):
                filler()

            def mm1(c, mt, t_ps):
                nc.tensor.matmul(
                    t_ps[:],
                    lhsT=a_sb[:, mt, :],
                    rhs=xsb[(c, mt // 16)][:, (mt // 2) % NPH, ds((mt % 2) * CH, CH)],
                    start=(mt == 0),
                    stop=(mt == N_MT - 1),
                    skip_group_check=True,
                )

            def mm1_chain(c):
                t_ps = tps_pool.tile([128, CH], F32, tag="tps", name="tps")
                for mt in range(N_MT):
                    mm1(c, mt, t_ps)
                t_sb = tpool.tile([128, CH], F16, tag="t", name="t")
                nc.vector.tensor_copy(t_sb[:], t_ps[:])
                return t_sb

            evac = 0
            t_sb_next = mm1_chain(0)

            for c in range(NCH):
                t_sb = t_sb_next

                for slot in range(NSTRIP * N):
                    s, n = divmod(slot, N)
                    if c == 0 and slot == 0:
                        filler()
                        filler()
                    osb = opool.tile([128, K], F16, tag="o", name="osb")
                    for kg in range(K // OPW):
                        o_ps = ops_pool.tile([128, OPW], F32, tag="ops", name="ops")
                        for kk in range(OPW // KT):
                            nc.tensor.matmul(
                                o_ps[:, ts(kk, KT)],
                                lhsT=t_sb[ds(ADP * n, D), ts(s, 128)],
                                rhs=b_sb[ds(ADP * n, D), ds(kg * OPW + kk * KT, KT)],
                                start=True,
                                stop=True,
                                tile_position=(ADP * n, 0),
                                skip_group_check=True,
                            )
                        if evac % 2 == 0:
                            nc.vector.tensor_copy(osb[:, ts(kg, OPW)], o_ps[:])
                        else:
                            nc.scalar.copy(osb[:, ts(kg, OPW)], o_ps[:])
                        evac += 1

                    nc.sync.dma_start(
                        o[n, ds(c * CH + s * 128, 128), :], osb[:]
                    )
                    # the next chunk's mm1 chain right after the first slot:
                    # late enough that its x has landed, early enough that
                    # the back-to-back chain re-warms the HAM mid-chunk
                    if slot == 0 and c + 1 < NCH:
                        t_sb_next = mm1_chain(c + 1)

    nc.compile()
    return nc


_NC_CACHE = []


def _get_nc():
    if not _NC_CACHE:
        _NC_CACHE.append(build_program())
    return _NC_CACHE[0]


def prepare_inputs(x, lora_A, lora_B):
    x = np.ascontiguousarray(np.asarray(x, dtype=np.float32)).astype(np.float16)
    lora_A = np.asarray(lora_A, dtype=np.float32)
    lora_B = np.asarray(lora_B, dtype=np.float32)

    xf = x.reshape(TOK, M)

    # a_t[m, 32n+d] = lora_A[n, d, m]; packed to [p, mt, c] so each SBUF
    # partition reads one contiguous row.
    a_t = np.zeros((M, 128), dtype=np.float32)
    for n in range(N):
        a_t[:, ADP * n : ADP * n + D] = lora_A[n].T
    a_pack = np.ascontiguousarray(
        a_t.reshape(N_MT, 128, 128).transpose(1, 0, 2)
    ).astype(np.float16)

    # b_pad[32n+d, k] = lora_B[n, k, d]
    b_pad = np.zeros((128, K), dtype=np.float16)
    for n in range(N):
        b_pad[ADP * n : ADP * n + D, :] = lora_B[n].T

    in_maps = []
    for c in range(N_CORES):
        # xp[chunk, half, p, pq, sub*CH + t] = x^T[(2*(8h+pq)+sub)*128 + p,
        #                                          chunk*CH + t]
        xT = xf[c * TPC : (c + 1) * TPC].T                  # [M, TPC]
        xr = xT.reshape(2, NPH, 2, 128, NCH, CH)            # [h, pq, sub, p, ch, t]
        xp = np.ascontiguousarray(xr.transpose(4, 0, 3, 1, 2, 5)).reshape(
            NCH, 2, 128, NPH, 2 * CH
        )
        in_maps.append({"xs": xp, "a_p": a_pack, "b_p": b_pad})
    return in_maps


def run(x, lora_A, lora_B, trace=False, **spmd_kwargs):
    nc = _get_nc()
    in_maps = prepare_inputs(x, lora_A, lora_B)
    res = bass_utils.run_bass_kernel_spmd(
        nc, in_maps, list(range(N_CORES)), trace=trace, **spmd_kwargs
    )
    o_full = np.concatenate(
        [res.results[c]["o"].astype(np.float32) for c in range(N_CORES)], axis=1
    )
    return o_full.reshape(N, B, J, K), res


def kernel(x, lora_A, lora_B):
    out, _ = run(x, lora_A, lora_B)
    return out
